# revision 1
# baseline (speedup 1.0000x reference)
"""CrossAttention TRN2 Bass kernel — 8-core data-parallel (batch x query-half).

Sharding: core c -> batch b=c//2, query rows [(c%2)*1024, (c%2+1)*1024).
Each core computes its 1024 output rows end-to-end (kv recomputed per
core-pair; no collectives). Host pre-transposes activations so every
matmul operand is contraction-major in DRAM.

kv compaction: masked kv positions are gathered out on the host (pad to
M2C=640 rows of zeros). Zero k rows give exp(0)=1 at pads, but the
ones-column appended to v carries the keep flag, so pads contribute
exactly 0 to both the attention numerator and the softmax denominator —
no mask bias or mask multiply anywhere on-chip.

Matmuls run in float32r (TF32-like, 1 cycle/row vs 4 for fp32; measured
~1.3e-4 scale-relative error per 1024-deep matmul). Scores are built
transposed (S^T[m,n]) so softmax reduction happens via the PE ones-column
trick; normalization runs in full fp32 (reciprocal + partition broadcast).
"""

import sys

sys.path.insert(0, "/opt/trn_rl_repo")

from contextlib import ExitStack

import numpy as np

import concourse.bass as bass
import concourse.tile as tile
from concourse import bacc, mybir
from concourse.bass_utils import run_bass_kernel_spmd

B, N, N2 = 4, 2048, 1024
DIM, H, HD = 1024, 16, 64
SCALE = HD ** -0.5
P = 128
R = 1024          # query rows per core
NCORES = 8
KO = DIM // P     # 8 contraction chunks
F32 = mybir.dt.float32
DT_MM = mybir.dt.float32r

M2C = 640         # compacted kv length (5 x 128); kept count must fit
MO = M2C // P

TRACE = False


def _mo_groups(mo):
    """Split mo chunks into groups of 3 then 2 for large ACT ops."""
    groups, i = [], 0
    while mo - i >= 3:
        groups.append((i, 3)); i += 3
    while mo - i > 0:
        g = min(2, mo - i)
        groups.append((i, g)); i += g
    return groups


def build_kernel(m2c=M2C):
    mo_n = m2c // P
    nc = bacc.Bacc("TRN2", target_bir_lowering=False, debug=False,
                   num_devices=NCORES)
    xT = nc.dram_tensor("xT", [DIM, R], F32, kind="ExternalInput").ap()
    yT = nc.dram_tensor("yT", [DIM, m2c], F32, kind="ExternalInput").ap()
    wq = nc.dram_tensor("wq", [DIM, DIM], F32, kind="ExternalInput").ap()
    wk = nc.dram_tensor("wk", [DIM, DIM], F32, kind="ExternalInput").ap()
    wv = nc.dram_tensor("wv", [DIM, DIM], F32, kind="ExternalInput").ap()
    wp = nc.dram_tensor("wp", [DIM, DIM], F32, kind="ExternalInput").ap()
    keepc = nc.dram_tensor("keepc", [m2c], F32, kind="ExternalInput").ap()
    bp = nc.dram_tensor("bp", [DIM], F32, kind="ExternalInput").ap()
    out = nc.dram_tensor("out", [DIM, R], F32, kind="ExternalOutput").ap()

    # kv free-dim chunking for the k^T projection (chunks must be >=256 for
    # full-rate float32r, <=512 for one PSUM bank)
    kv_chunks = [(i, min(320, m2c - i)) for i in range(0, m2c, 320)]

    with tile.TileContext(nc, pool_alloc_mode="queue") as tc, ExitStack() as ctx:
        persist = ctx.enter_context(tc.tile_pool(name="persist", bufs=1))
        qT = persist.tile([P, KO, R], DT_MM)        # q^T, c-major
        kT = persist.tile([P, KO, m2c], DT_MM)      # k^T, c-major
        vS = persist.tile([P, mo_n, H * 65], DT_MM)  # v[m,c] + keep col / head
        attnT = persist.tile([P, KO, R], DT_MM)     # attn out^T, c-major
        kc = persist.tile([P, mo_n], F32)           # keep col, m-major
        bT = persist.tile([P, KO], F32)

        nc.sync.dma_start(kc[:], keepc.rearrange("(mo p) -> p mo", p=P))
        nc.sync.dma_start(bT[:], bp.rearrange("(o p) -> p o", p=P))
        vH = vS.rearrange("p mo (h s) -> p mo h s", s=65)
        for mo in range(mo_n):
            nc.vector.tensor_copy(vH[:, mo, :, 64],
                                  kc[:, mo:mo + 1].to_broadcast([P, H]))

        def load_cast(stg_pool, dram2d, dest, fdim, tag):
            """DRAM [DIM, fdim] -> dest [P, KO, fdim] (DT_MM via DVE round)."""
            src3 = dram2d.rearrange("(ko p) f -> p ko f", p=P)
            for ko in range(KO):
                stg = stg_pool.tile([P, fdim], F32, name=f"stg_{tag}_{ko}",
                                    tag=f"stg_{tag}")
                nc.sync.dma_start(stg[:], src3[:, ko])
                nc.vector.tensor_copy(dest[:, ko], stg[:])

        # ---- Phase A: yT load hoisted so it overlaps A1 compute ----
        with tc.tile_pool(name="pY", bufs=1) as py, \
             tc.tile_pool(name="stgY", bufs=2) as stgy:
            yT_r = py.tile([P, KO, m2c], DT_MM)
            load_cast(stgy, yT, yT_r, m2c, "yt")

            with tc.tile_pool(name="pA1", bufs=1) as pa1, \
                 tc.tile_pool(name="stgA1", bufs=2) as stg1, \
                 tc.tile_pool(name="wqsl", bufs=2) as wqp, \
                 tc.tile_pool(name="psA1", bufs=4, space="PSUM") as psa1:
                xT_r = pa1.tile([P, KO, R], DT_MM)
                load_cast(stg1, xT, xT_r, R, "xt")
                wq3 = wq.rearrange("(ko p) c -> p ko c", p=P)
                for co in range(KO):
                    wstg = wqp.tile([P, KO, P], F32, tag="wqstg")
                    nc.sync.dma_start(wstg[:], wq3[:, :, co * P:(co + 1) * P])
                    wsl = wqp.tile([P, KO, P], DT_MM, tag="wqr")
                    nc.vector.tensor_copy(wsl[:], wstg[:])
                    for nn in range(2):
                        ps = psa1.tile([P, 512], F32, tag="psq")
                        for ko in range(KO):
                            nc.tensor.matmul(
                                ps[:], wsl[:, ko], xT_r[:, ko, nn * 512:(nn + 1) * 512],
                                start=(ko == 0), stop=(ko == KO - 1))
                        nc.vector.tensor_copy(qT[:, co, nn * 512:(nn + 1) * 512], ps[:])

            with tc.tile_pool(name="wksl", bufs=2) as wkp, \
                 tc.tile_pool(name="psA2", bufs=4, space="PSUM") as psa2:
                wk3 = wk.rearrange("(ko p) c -> p ko c", p=P)
                for co in range(KO):
                    wstg = wkp.tile([P, KO, P], F32, tag="wkstg")
                    nc.sync.dma_start(wstg[:], wk3[:, :, co * P:(co + 1) * P])
                    wsl = wkp.tile([P, KO, P], DT_MM, tag="wkr")
                    nc.vector.tensor_copy(wsl[:], wstg[:])
                    for m0, mw in kv_chunks:
                        ps = psa2.tile([P, 320], F32, tag="psk")
                        for ko in range(KO):
                            nc.tensor.matmul(
                                ps[:, :mw], wsl[:, ko], yT_r[:, ko, m0:m0 + mw],
                                start=(ko == 0), stop=(ko == KO - 1))
                        nc.vector.tensor_copy(kT[:, co, m0:m0 + mw], ps[:, :mw])

            with tc.tile_pool(name="wvsl", bufs=2) as wvp, \
                 tc.tile_pool(name="psA3", bufs=4, space="PSUM") as psa3:
                wv3 = wv.rearrange("(ko p) c -> p ko c", p=P)
                for c4 in range(4):  # 256-wide v column chunks (4 heads)
                    wstg = wvp.tile([P, KO, 256], F32, tag="wvstg")
                    nc.sync.dma_start(wstg[:], wv3[:, :, c4 * 256:(c4 + 1) * 256])
                    wsl = wvp.tile([P, KO, 256], DT_MM, tag="wvr")
                    nc.vector.tensor_copy(wsl[:], wstg[:])
                    for mo in range(mo_n):
                        ps = psa3.tile([P, 256], F32, tag="psv")
                        for ko in range(KO):
                            nc.tensor.matmul(
                                ps[:], yT_r[:, ko, mo * P:(mo + 1) * P],
                                wsl[:, ko],
                                start=(ko == 0), stop=(ko == KO - 1))
                        nc.vector.tensor_copy(
                            vH[:, mo, c4 * 4:(c4 + 1) * 4, 0:64],
                            ps.rearrange("p (h d) -> p h d", d=64))

        # ---- Phase B: attention. nn outer so phase C can start early ----
        groups = _mo_groups(mo_n)
        with tc.tile_pool(name="pBe", bufs=2) as pbe, \
             tc.tile_pool(name="psS", bufs=2, space="PSUM") as pss, \
             tc.tile_pool(name="psO", bufs=2, space="PSUM") as pso:
            for nn in range(2):
                for h in range(H):
                    pq = (h % 2) * 64
                    co = h // 2
                    expS = pbe.tile([P, mo_n, 512], DT_MM, tag="expS")
                    for g0, gn in groups:
                        sps = pss.tile([P, 3, 512], F32, tag="sps")
                        for mo in range(g0, g0 + gn):
                            nc.tensor.matmul(
                                sps[:, mo - g0],
                                kT[pq:pq + 64, co, mo * P:(mo + 1) * P],
                                qT[pq:pq + 64, co, nn * 512:(nn + 1) * 512],
                                start=True, stop=True)
                        nc.scalar.activation(
                            expS[:, g0:g0 + gn], sps[:, :gn],
                            mybir.ActivationFunctionType.Exp,
                            scale=float(SCALE))
                    ops = pso.tile([P, 512], F32, tag="ops")
                    for mo in range(mo_n):
                        nc.tensor.matmul(
                            ops[0:65], vS[:, mo, h * 65:(h + 1) * 65],
                            expS[:, mo],
                            start=(mo == 0), stop=(mo == mo_n - 1))
                    rec = pbe.tile([1, 512], F32, tag="rec")
                    nc.vector.reciprocal(rec[:], ops[64:65])
                    bc = pbe.tile([64, 512], F32, tag="bc")
                    nc.gpsimd.partition_broadcast(bc[:], rec[:])
                    nc.vector.tensor_mul(
                        attnT[pq:pq + 64, co, nn * 512:(nn + 1) * 512],
                        ops[0:64], bc[:])

        # ---- Phase C: outT[c2,n] = Wproj^T-major proj + bias ----
        with tc.tile_pool(name="pWp", bufs=1) as pwp, \
             tc.tile_pool(name="stgWp", bufs=2) as stgwp, \
             tc.tile_pool(name="outp", bufs=3) as outp, \
             tc.tile_pool(name="psC", bufs=4, space="PSUM") as psc:
            wp_r = pwp.tile([P, KO, DIM], DT_MM)
            load_cast(stgwp, wp, wp_r, DIM, "wp")
            for nn in range(2):
                for c2o in range(KO):
                    ps = psc.tile([P, 512], F32, tag="psc")
                    for co in range(KO):
                        nc.tensor.matmul(
                            ps[:], wp_r[:, co, c2o * P:(c2o + 1) * P],
                            attnT[:, co, nn * 512:(nn + 1) * 512],
                            start=(co == 0), stop=(co == KO - 1))
                    osb = outp.tile([P, 512], F32, tag="osb")
                    nc.vector.tensor_scalar_add(osb[:], ps[:], bT[:, c2o:c2o + 1])
                    nc.sync.dma_start(
                        out[c2o * P:(c2o + 1) * P, nn * 512:(nn + 1) * 512], osb[:])

    nc.finalize()
    return nc


_NC = {}


def kernel(x, y, pad_mask, Wq, Wkv, Wproj, bproj):
    x = np.asarray(x, dtype=np.float32)
    y = np.asarray(y, dtype=np.float32)
    pad_mask = np.asarray(pad_mask)
    Wq = np.ascontiguousarray(np.asarray(Wq, dtype=np.float32))
    Wkv = np.asarray(Wkv, dtype=np.float32)
    Wproj = np.ascontiguousarray(np.asarray(Wproj, dtype=np.float32))
    bproj = np.asarray(bproj, dtype=np.float32)

    Wk = np.ascontiguousarray(Wkv[:, :DIM])
    Wv = np.ascontiguousarray(Wkv[:, DIM:])

    # compact kv: gather kept rows per batch, pad with zeros to m2c
    keep_idx = [np.nonzero(pad_mask[b] != 0)[0] for b in range(B)]
    max_kept = max(len(i) for i in keep_idx)
    m2c = M2C if max_kept <= M2C else N2
    yc = np.zeros((B, m2c, DIM), dtype=np.float32)
    keepc = np.zeros((B, m2c), dtype=np.float32)
    for b in range(B):
        k = len(keep_idx[b])
        yc[b, :k] = y[b][keep_idx[b]]
        keepc[b, :k] = 1.0

    in_maps = []
    for c in range(NCORES):
        b, half = c // 2, c % 2
        in_maps.append({
            "xT": np.ascontiguousarray(x[b, half * R:(half + 1) * R, :].T),
            "yT": np.ascontiguousarray(yc[b].T),
            "wq": Wq, "wk": Wk, "wv": Wv, "wp": Wproj,
            "keepc": keepc[b],
            "bp": bproj,
        })

    if m2c not in _NC:
        _NC[m2c] = build_kernel(m2c)

    res = run_bass_kernel_spmd(_NC[m2c], in_maps, core_ids=list(range(NCORES)),
                               trace=TRACE)
    if TRACE:
        kernel.last_results = res

    full = np.empty((B, N, DIM), dtype=np.float32)
    for c in range(NCORES):
        b, half = c // 2, c % 2
        full[b, half * R:(half + 1) * R, :] = res.results[c]["out"].T
    return full



# revision 3
# speedup vs baseline: 1.1980x; 1.1980x over previous
"""CrossAttention TRN2 Bass kernel — 8-core data-parallel (batch x query-half).

Sharding: core c -> batch b=c//2, query rows [(c%2)*1024, (c%2+1)*1024).
Each core computes its 1024 output rows end-to-end (kv recomputed per
core-pair; no collectives). Host pre-transposes activations so every
matmul operand is contraction-major in DRAM.

kv compaction: masked kv positions are gathered out on the host (pad to
M2C=640 rows of zeros). Zero k rows give exp(0)=1 at pads, but the
ones-column appended to v carries the keep flag, so pads contribute
exactly 0 to both the attention numerator and the softmax denominator —
no mask bias or mask multiply anywhere on-chip.

All matmul operands are bf16, cast on the HOST (ml_dtypes): halves HBM
traffic vs fp32, removes every on-chip weight cast, and draws less PE
power than fp32r (the fp32r version tripped the HW power throttle to a
~60% util limit for the back 2/3 of the kernel). PSUM accumulation is
fp32 throughout, softmax normalization runs in fp32 (fast approx
reciprocal, ~18 good bits) — measured scale-rel error ~1e-3 vs the 2e-2
gate.

Scores are built transposed (S^T[m,n]) so the softmax denominator comes
free via a ones-column appended to v in the attn@v matmul.
"""

import sys

sys.path.insert(0, "/opt/trn_rl_repo")

from contextlib import ExitStack

import ml_dtypes
import numpy as np

import concourse.bass as bass
import concourse.tile as tile
from concourse import bacc, mybir
from concourse.bass_utils import run_bass_kernel_spmd

B, N, N2 = 4, 2048, 1024
DIM, H, HD = 1024, 16, 64
SCALE = HD ** -0.5
P = 128
R = 1024          # query rows per core
NCORES = 8
KO = DIM // P     # 8 contraction chunks
F32 = mybir.dt.float32
BF = mybir.dt.bfloat16
NPBF = ml_dtypes.bfloat16

M2C = 640         # compacted kv length (5 x 128); kept count must fit
MO = M2C // P

TRACE = False


def _mo_groups(mo):
    """Split mo chunks into groups of 3 then 2 for large ACT ops."""
    groups, i = [], 0
    while mo - i >= 3:
        groups.append((i, 3)); i += 3
    while mo - i > 0:
        g = min(2, mo - i)
        groups.append((i, g)); i += g
    return groups


def build_kernel(m2c=M2C):
    mo_n = m2c // P
    nc = bacc.Bacc("TRN2", target_bir_lowering=False, debug=False,
                   num_devices=NCORES)
    xT = nc.dram_tensor("xT", [DIM, R], BF, kind="ExternalInput").ap()
    yT = nc.dram_tensor("yT", [DIM, m2c], BF, kind="ExternalInput").ap()
    wq = nc.dram_tensor("wq", [DIM, DIM], BF, kind="ExternalInput").ap()
    wk = nc.dram_tensor("wk", [DIM, DIM], BF, kind="ExternalInput").ap()
    wv = nc.dram_tensor("wv", [DIM, DIM], BF, kind="ExternalInput").ap()
    wp = nc.dram_tensor("wp", [DIM, DIM], BF, kind="ExternalInput").ap()
    keepc = nc.dram_tensor("keepc", [m2c], BF, kind="ExternalInput").ap()
    bp = nc.dram_tensor("bp", [DIM], F32, kind="ExternalInput").ap()
    out = nc.dram_tensor("out", [DIM, R], F32, kind="ExternalOutput").ap()

    # kv free-dim chunking for the k^T projection (<=512 for one PSUM bank)
    kv_chunks = [(i, min(320, m2c - i)) for i in range(0, m2c, 320)]

    with tile.TileContext(nc, pool_alloc_mode="queue") as tc, ExitStack() as ctx:
        persist = ctx.enter_context(tc.tile_pool(name="persist", bufs=1))
        qT = persist.tile([P, KO, R], BF)           # q^T, c-major
        kT = persist.tile([P, KO, m2c], BF)         # k^T, c-major
        vS = persist.tile([P, mo_n, H * 65], BF)    # v[m,c] + keep col / head
        attnT = persist.tile([P, KO, R], BF)        # attn out^T, c-major
        wp_r = persist.tile([P, KO, DIM], BF)       # Wproj, loaded during B
        kc = persist.tile([P, mo_n], BF)            # keep col, m-major
        bT = persist.tile([P, KO], F32)

        wq3 = wq.rearrange("(ko p) c -> p ko c", p=P)
        wk3 = wk.rearrange("(ko p) c -> p ko c", p=P)
        wv3 = wv.rearrange("(ko p) c -> p ko c", p=P)
        wp3 = wp.rearrange("(ko p) c -> p ko c", p=P)

        # ---- Phase A1: q = x @ Wq. First DMAs issued = first compute. ----
        with tc.tile_pool(name="pA1", bufs=1) as pa1, \
             tc.tile_pool(name="wqsl", bufs=2) as wqp, \
             tc.tile_pool(name="pY", bufs=1) as py, \
             tc.tile_pool(name="psA1", bufs=4, space="PSUM") as psa1:
            xT_r = pa1.tile([P, KO, R], BF)
            xr3 = xT.rearrange("(ko p) f -> p ko f", p=P)
            # queue the first weight chunk alongside the activations
            wsl0 = wqp.tile([P, KO, 256], BF, tag="wq")
            nc.sync.dma_start(wsl0[:], wq3[:, :, 0:256])
            for ko in range(KO):
                nc.sync.dma_start(xT_r[:, ko], xr3[:, ko])
            nc.sync.dma_start(kc[:], keepc.rearrange("(mo p) -> p mo", p=P))
            nc.sync.dma_start(bT[:], bp.rearrange("(o p) -> p o", p=P))
            vH = vS.rearrange("p mo (h s) -> p mo h s", s=65)
            for mo in range(mo_n):
                nc.vector.tensor_copy(vH[:, mo, :, 64],
                                      kc[:, mo:mo + 1].to_broadcast([P, H]))
            # yT lands behind wq chunks; needed only when A2 starts
            yT_r = py.tile([P, KO, m2c], BF)
            yr3 = yT.rearrange("(ko p) f -> p ko f", p=P)

            for cq in range(4):
                wsl = wsl0 if cq == 0 else wqp.tile([P, KO, 256], BF, tag="wq")
                if cq > 0:
                    nc.sync.dma_start(wsl[:], wq3[:, :, cq * 256:(cq + 1) * 256])
                if cq == 1:
                    for ko in range(KO):
                        nc.sync.dma_start(yT_r[:, ko], yr3[:, ko])
                for c2 in range(2):
                    co = cq * 2 + c2
                    for nn in range(2):
                        ps = psa1.tile([P, 512], F32, tag="psq")
                        for ko in range(KO):
                            nc.tensor.matmul(
                                ps[:], wsl[:, ko, c2 * P:(c2 + 1) * P],
                                xT_r[:, ko, nn * 512:(nn + 1) * 512],
                                start=(ko == 0), stop=(ko == KO - 1))
                        nc.vector.tensor_copy(qT[:, co, nn * 512:(nn + 1) * 512],
                                              ps[:])

            # ---- Phase A2: k^T = Wk^T @ y^T ----
            with tc.tile_pool(name="wksl", bufs=2) as wkp, \
                 tc.tile_pool(name="psA2", bufs=4, space="PSUM") as psa2:
                for cq in range(4):
                    wsl = wkp.tile([P, KO, 256], BF, tag="wk")
                    nc.sync.dma_start(wsl[:], wk3[:, :, cq * 256:(cq + 1) * 256])
                    for c2 in range(2):
                        co = cq * 2 + c2
                        for m0, mw in kv_chunks:
                            ps = psa2.tile([P, 320], F32, tag="psk")
                            for ko in range(KO):
                                nc.tensor.matmul(
                                    ps[:, :mw], wsl[:, ko, c2 * P:(c2 + 1) * P],
                                    yT_r[:, ko, m0:m0 + mw],
                                    start=(ko == 0), stop=(ko == KO - 1))
                            nc.vector.tensor_copy(kT[:, co, m0:m0 + mw],
                                                  ps[:, :mw])

            # ---- Phase A3: v = y @ Wv (m-major, heads split, keep col) ----
            with tc.tile_pool(name="wvsl", bufs=2) as wvp, \
                 tc.tile_pool(name="psA3", bufs=4, space="PSUM") as psa3:
                for c4 in range(4):  # 256-wide v column chunks (4 heads)
                    wsl = wvp.tile([P, KO, 256], BF, tag="wv")
                    nc.sync.dma_start(wsl[:], wv3[:, :, c4 * 256:(c4 + 1) * 256])
                    for mo in range(mo_n):
                        ps = psa3.tile([P, 256], F32, tag="psv")
                        for ko in range(KO):
                            nc.tensor.matmul(
                                ps[:], yT_r[:, ko, mo * P:(mo + 1) * P],
                                wsl[:, ko],
                                start=(ko == 0), stop=(ko == KO - 1))
                        nc.vector.tensor_copy(
                            vH[:, mo, c4 * 4:(c4 + 1) * 4, 0:64],
                            ps.rearrange("p (h d) -> p h d", d=64))

        # ---- Phase B: attention. nn outer so phase C can start early ----
        # Wproj streams in under B's compute shadow.
        for cw in range(4):
            nc.sync.dma_start(wp_r[:, :, cw * 256:(cw + 1) * 256],
                              wp3[:, :, cw * 256:(cw + 1) * 256])
        groups = _mo_groups(mo_n)
        with tc.tile_pool(name="pBe", bufs=2) as pbe, \
             tc.tile_pool(name="psS", bufs=2, space="PSUM") as pss, \
             tc.tile_pool(name="psO", bufs=2, space="PSUM") as pso:
            for nn in range(2):
                for h in range(H):
                    pq = (h % 2) * 64
                    co = h // 2
                    expS = pbe.tile([P, mo_n, 512], BF, tag="expS")
                    for g0, gn in groups:
                        sps = pss.tile([P, 3, 512], F32, tag="sps")
                        for mo in range(g0, g0 + gn):
                            nc.tensor.matmul(
                                sps[:, mo - g0],
                                kT[pq:pq + 64, co, mo * P:(mo + 1) * P],
                                qT[pq:pq + 64, co, nn * 512:(nn + 1) * 512],
                                start=True, stop=True)
                        nc.scalar.activation(
                            expS[:, g0:g0 + gn], sps[:, :gn],
                            mybir.ActivationFunctionType.Exp,
                            scale=float(SCALE))
                    ops = pso.tile([P, 512], F32, tag="ops")
                    for mo in range(mo_n):
                        nc.tensor.matmul(
                            ops[0:65], vS[:, mo, h * 65:(h + 1) * 65],
                            expS[:, mo],
                            start=(mo == 0), stop=(mo == mo_n - 1))
                    den = pbe.tile([1, 512], F32, tag="den")
                    nc.vector.tensor_copy(den[:], ops[64:65])
                    rec = pbe.tile([1, 512], F32, tag="rec")
                    # denom must bounce through SBUF: the custom-DVE recip
                    # reads garbage from PSUM (probed on HW)
                    nc.vector.reciprocal_approx_fast(rec[:], den[:])
                    bc = pbe.tile([64, 512], F32, tag="bc")
                    nc.gpsimd.partition_broadcast(bc[:], rec[:])
                    nc.vector.tensor_mul(
                        attnT[pq:pq + 64, co, nn * 512:(nn + 1) * 512],
                        ops[0:64], bc[:])

        # ---- Phase C: outT[c2,n] = Wproj^T-major proj + bias ----
        with tc.tile_pool(name="outp", bufs=3) as outp, \
             tc.tile_pool(name="psC", bufs=4, space="PSUM") as psc:
            for nn in range(2):
                for c2o in range(KO):
                    ps = psc.tile([P, 512], F32, tag="psc")
                    for co in range(KO):
                        nc.tensor.matmul(
                            ps[:], wp_r[:, co, c2o * P:(c2o + 1) * P],
                            attnT[:, co, nn * 512:(nn + 1) * 512],
                            start=(co == 0), stop=(co == KO - 1))
                    osb = outp.tile([P, 512], F32, tag="osb")
                    nc.vector.tensor_scalar_add(osb[:], ps[:], bT[:, c2o:c2o + 1])
                    nc.sync.dma_start(
                        out[c2o * P:(c2o + 1) * P, nn * 512:(nn + 1) * 512], osb[:])

    nc.finalize()
    return nc


_NC = {}


def kernel(x, y, pad_mask, Wq, Wkv, Wproj, bproj):
    x = np.asarray(x, dtype=np.float32)
    y = np.asarray(y, dtype=np.float32)
    pad_mask = np.asarray(pad_mask)
    Wq = np.asarray(Wq, dtype=np.float32)
    Wkv = np.asarray(Wkv, dtype=np.float32)
    Wproj = np.asarray(Wproj, dtype=np.float32)
    bproj = np.asarray(bproj, dtype=np.float32)

    Wqb = np.ascontiguousarray(Wq.astype(NPBF))
    Wkb = np.ascontiguousarray(Wkv[:, :DIM].astype(NPBF))
    Wvb = np.ascontiguousarray(Wkv[:, DIM:].astype(NPBF))
    Wpb = np.ascontiguousarray(Wproj.astype(NPBF))

    # compact kv: gather kept rows per batch, pad with zeros to m2c
    keep_idx = [np.nonzero(pad_mask[b] != 0)[0] for b in range(B)]
    max_kept = max(len(i) for i in keep_idx)
    m2c = M2C if max_kept <= M2C else N2
    yc = np.zeros((B, m2c, DIM), dtype=np.float32)
    keepc = np.zeros((B, m2c), dtype=NPBF)
    for b in range(B):
        k = len(keep_idx[b])
        yc[b, :k] = y[b][keep_idx[b]]
        keepc[b, :k] = 1.0

    xTb = [np.ascontiguousarray(x[b, half * R:(half + 1) * R, :].T.astype(NPBF))
           for b in range(B) for half in range(2)]
    yTb = [np.ascontiguousarray(yc[b].T.astype(NPBF)) for b in range(B)]

    in_maps = []
    for c in range(NCORES):
        b, half = c // 2, c % 2
        in_maps.append({
            "xT": xTb[c],
            "yT": yTb[b],
            "wq": Wqb, "wk": Wkb, "wv": Wvb, "wp": Wpb,
            "keepc": keepc[b],
            "bp": bproj,
        })

    if m2c not in _NC:
        _NC[m2c] = build_kernel(m2c)

    res = run_bass_kernel_spmd(_NC[m2c], in_maps, core_ids=list(range(NCORES)),
                               trace=TRACE)
    if TRACE:
        kernel.last_results = res

    full = np.empty((B, N, DIM), dtype=np.float32)
    for c in range(NCORES):
        b, half = c // 2, c % 2
        full[b, half * R:(half + 1) * R, :] = res.results[c]["out"].T
    return full
